# revision 12
# baseline (speedup 1.0000x reference)
"""Single-head causal attention (V=K source bug) on 8 trn2 NeuronCores. v2.

Problem: x[4,2048,1024], W_Q/W_K/W_V[64,1024] (W_V unused by reference).
  Q = x @ W_Q.T ; K = x @ W_K.T ; V = K (reference bug)
  out = softmax(mask(Q K^T / sqrt(1024))) @ V      -> [4,2048,64]

Sharding: 2 cores per batch (core i: batch = i % 4, role r = i // 4).
Each batch's 8 query tiles of 256 rows split by parity (r=0 even, r=1 odd).
ONE SPMD graph; per-core differences folded into data:
 * x^T sent column-PERMUTED (own tiles first); causality over the permuted
   key order is uniform: own chunk 2s+h at slot s masks p+128h<=f
   (device affine_select, no mask DMA); oth chunks (2s,2s+1) at slot s
   are all-valid (r=1) or all-masked (r=0) -> scal 0/1 multiply.

v2 layout (vs v1): single sync HWDGE queue, dep-ordered col-half DMAs;
qT duplicated into rows 0-63 (qlow pass) and 64-127 (joint K|Q pass,
col-tiled) with no SBUF-SBUF dup DMA; kstk rows 0-63 = own K^T chunks,
rows 64-127 = oth (tile_position col packing); one [128,128] PE transpose
per chunk column yields V for own+oth together; outputs on the sync HW
queue. ScalarE runs only the 13-ACT exp chain.
"""

import sys

sys.path.insert(0, "/opt/trn_rl_repo")

import numpy as np
import ml_dtypes

BF16 = ml_dtypes.bfloat16

B, T, C, D = 4, 2048, 1024, 64
N_CORES = 8
QTILE = 256
N_SLOTS = 4
CHUNK = 128
SCALE = C ** -0.5
N_WARMUP = 44

TRACE = False
TRACE_CORES = None
LAST_RESULTS = None

# ---- unit table ----------------------------------------------------------
# cell = (side, chunk, slot): side 0 = own (kstk rows 0:64, rhs qT[0:64]),
# side 1 = oth (rows 64:128, rhs qT[64:128]).
# Units W1..W10: 4 slices of [128,256] in one PSUM tile [128, 4, 256].
# Each entry: (name, slices[4] of cell, exp_halves, diag, rmask)
#   exp_halves: list of (slice_lo, slice_hi, tier) - ACTIVATE granularity
#   diag: (slice_lo,) one affine_select over slices [lo, lo+2) or None
#   rmask: list of (slice_lo, n_slices) tensor_scalar muls
# tiers (DMA arrival): 0=s0a 1=s0b 2=s1 3=s2 4=s3


def _unit_defs():
    U = []
    U.append(("W1", [(0, 0, 0), (0, 1, 0), (0, 0, 1), (0, 1, 1)],
              [(0, 2, 0), (2, 4, 1)], 0, []))
    U.append(("W2", [(0, 2, 1), (0, 3, 1), (0, 2, 2), (0, 3, 2)],
              [(0, 2, 1), (2, 4, 2)], 0, []))
    U.append(("W3", [(0, 0, 2), (0, 1, 2), (0, 4, 2), (0, 5, 2)],
              [(0, 4, 2)], 2, []))
    U.append(("W4", [(0, 0, 3), (0, 1, 3), (0, 2, 3), (0, 3, 3)],
              [(0, 4, 2)], None, []))
    U.append(("W5", [(0, 4, 3), (0, 5, 3), (0, 6, 3), (0, 7, 3)],
              [(0, 4, 2)], 2, []))
    U.append(("W6", [(1, 0, 0), (1, 0, 1), (1, 1, 0), (1, 1, 1)],
              [(0, 4, 3)], None, [(0, 1), (2, 1)]))
    U.append(("W7", [(1, 0, 2), (1, 0, 3), (1, 1, 2), (1, 1, 3)],
              [(0, 4, 3)], None, []))
    U.append(("W8", [(1, 2, 1), (1, 2, 2), (1, 3, 1), (1, 3, 2)],
              [(0, 4, 3)], None, [(0, 1), (2, 1)]))
    U.append(("W9", [(1, 4, 2), (1, 4, 3), (1, 5, 2), (1, 5, 3)],
              [(0, 4, 4)], None, [(0, 1), (2, 1)]))
    U.append(("W10", [(1, 2, 3), (1, 3, 3), (1, 6, 3), (1, 7, 3)],
              [(0, 2, 3), (2, 4, 4)], None, [(2, 2)]))
    return U


# S^T matmuls per unit: list of (slice_lo, n_slices, side, chunk, slot_lo)
# wide (n_slices=2) only when the slices are the same chunk at adjacent
# slots AND arrive in one tier.
_ST_MMS = {
    "W1": [(0, 1, 0, 0, 0), (1, 1, 0, 1, 0), (2, 1, 0, 0, 1), (3, 1, 0, 1, 1)],
    "W2": [(0, 1, 0, 2, 1), (1, 1, 0, 3, 1), (2, 1, 0, 2, 2), (3, 1, 0, 3, 2)],
    "W3": [(0, 1, 0, 0, 2), (1, 1, 0, 1, 2), (2, 1, 0, 4, 2), (3, 1, 0, 5, 2)],
    "W4": [(0, 1, 0, 0, 3), (1, 1, 0, 1, 3), (2, 1, 0, 2, 3), (3, 1, 0, 3, 3)],
    "W5": [(0, 1, 0, 4, 3), (1, 1, 0, 5, 3), (2, 1, 0, 6, 3), (3, 1, 0, 7, 3)],
    "W6": [(0, 2, 1, 0, 0), (2, 2, 1, 1, 0)],
    "W7": [(0, 2, 1, 0, 2), (2, 2, 1, 1, 2)],
    "W8": [(0, 2, 1, 2, 1), (2, 2, 1, 3, 1)],
    "W9": [(0, 2, 1, 4, 2), (2, 2, 1, 5, 2)],
    "W10": [(0, 1, 1, 2, 3), (1, 1, 1, 3, 3), (2, 1, 1, 6, 3), (3, 1, 1, 7, 3)],
}


def _build_graph():
    import concourse.bass as bass
    import concourse.mybir as mybir
    import concourse.tile as tile
    from concourse import bacc
    from concourse.masks import make_identity
    from contextlib import ExitStack

    fp32 = mybir.dt.float32
    bf16 = mybir.dt.bfloat16
    EXP = mybir.ActivationFunctionType.Exp

    nc = bacc.Bacc(
        "TRN2",
        target_bir_lowering=False,
        debug=False,
        num_devices=N_CORES,
    )

    xkt = nc.dram_tensor("xkt", [C, T], bf16, kind="ExternalInput").ap()
    wkq = nc.dram_tensor("wkq", [C, 2 * D], bf16, kind="ExternalInput").ap()
    scald = nc.dram_tensor("scal", [CHUNK, 1], fp32, kind="ExternalInput").ap()
    out = nc.dram_tensor(
        "out", [D + 1, N_SLOTS * QTILE], fp32, kind="ExternalOutput"
    ).ap()

    CCH = C // CHUNK  # 8 contraction chunks
    units = _unit_defs()

    with tile.TileContext(nc) as tc, ExitStack() as ctx:
        consts = ctx.enter_context(tc.tile_pool(name="consts", bufs=1))
        xpool = ctx.enter_context(tc.tile_pool(name="xpool", bufs=1))
        kqpool = ctx.enter_context(tc.tile_pool(name="kqpool", bufs=1))
        ptpool = ctx.enter_context(tc.tile_pool(name="ptpool", bufs=10))
        opool = ctx.enter_context(tc.tile_pool(name="opool", bufs=4))
        psS = ctx.enter_context(tc.tile_pool(name="psS", bufs=2, space="PSUM"))
        psP = ctx.enter_context(tc.tile_pool(name="psP", bufs=2, space="PSUM"))
        psO = ctx.enter_context(tc.tile_pool(name="psO", bufs=1, space="PSUM"))

        # ---- constants / statics ----
        warm_src = consts.tile([128, 512], bf16)
        nc.vector.memset(warm_src, 0.0)
        warm_ps = psP.tile([128, 512], fp32, tag="proj", name="warm_ps")
        for w in range(N_WARMUP):
            nc.tensor.matmul(
                warm_ps[:, 0:128], lhsT=warm_src[:, 0:128],
                rhs=warm_src[:, 0:128],
                start=(w == 0), stop=(w == N_WARMUP - 1),
            )
        ident = consts.tile([128, 128], bf16)
        make_identity(nc, ident)
        warm = consts.tile([1, 1], fp32)
        nc.vector.memset(warm, 0.0)
        nc.scalar.activation(warm, warm, EXP)

        w_sb = consts.tile([128, CCH, 2 * D], bf16)
        scal_sb = consts.tile([128, 1], fp32)
        xs = [xpool.tile([128, CCH, 512], bf16, name=f"xslab{s}")
              for s in range(4)]

        # ---- input DMAs: one sync HWDGE queue, dependency order ----
        nc.sync.dma_start(out=scal_sb, in_=scald)
        nc.sync.dma_start(out=w_sb, in_=wkq.rearrange("(c p) d -> p c d", p=128))
        xkt_r = xkt.rearrange("(c p) t -> p c t", p=128)

        for h in range(2):
            nc.sync.dma_start(
                out=xs[0][:, :, h * 256 : h * 256 + 256],
                in_=xkt_r[:, :, h * 256 : h * 256 + 256],
            )
        for s in range(1, 4):
            nc.sync.dma_start(
                out=xs[s], in_=xkt_r[:, :, s * 512 : (s + 1) * 512]
            )

        # ---- persistent SBUF tensors ----
        kstk = kqpool.tile([128, 1024], bf16)   # r0-63 own K^T, r64-127 oth
        qT = kqpool.tile([128, 1024], bf16)     # Q^T in both halves
        vones = kqpool.tile([128, CCH, 2, D + 2], bf16)
        nc.vector.memset(vones[:, :, :, D : D + 1], 1.0)

        # ---- engine-op emitters ----
        def joint_proj(s, c0, c1):
            """K|Q joint projection of cols [c0,c1) of slab s (slabs 0,1).
            rows 0-63 K^T -> kstk, rows 64-127 Q^T -> qT hi (both DVE)."""
            w = c1 - c0
            kq = psP.tile([128, 512], fp32, tag="proj", name=f"kq{s}{c0}")
            for c in range(CCH):
                nc.tensor.matmul(
                    kq[:, 0:w], lhsT=w_sb[:, c, :], rhs=xs[s][:, c, c0:c1],
                    start=(c == 0), stop=(c == CCH - 1),
                )
            dst = slice(s * 512 + c0, s * 512 + c1)
            nc.vector.tensor_copy(kstk[0:64, dst], kq[0:64, 0:w])
            return kq

        def qthi(kq, s, c0, c1):
            """deferred Q^T hi copy (only oth-side S^T needs it)."""
            w = c1 - c0
            nc.vector.tensor_copy(
                qT[64:128, s * 512 + c0 : s * 512 + c1], kq[64:128, 0:w]
            )

        def qlow_proj(s, c0, c1):
            w = c1 - c0
            qp = psP.tile([128, 512], fp32, tag="proj", name=f"ql{s}{c0}")
            for c in range(CCH):
                nc.tensor.matmul(
                    qp[0:64, 0:w], lhsT=w_sb[:, c, D : 2 * D],
                    rhs=xs[s][:, c, c0:c1],
                    start=(c == 0), stop=(c == CCH - 1),
                )
            dst = slice(s * 512 + c0, s * 512 + c1)
            nc.vector.tensor_copy(qT[0:64, dst], qp[0:64, 0:w])

        def kproj_oth(s):
            """K^T of full slab s (2,3) -> kstk rows 64-127 (col-tiled via
            out base_partition 64)."""
            kp = psP.tile([128, 512], fp32, tag="proj", name=f"ko{s}")
            for c in range(CCH):
                nc.tensor.matmul(
                    kp[64:128, :], lhsT=w_sb[:, c, 0:D], rhs=xs[s][:, c, :],
                    start=(c == 0), stop=(c == CCH - 1),
                )
            d0 = (s - 2) * 512
            nc.vector.tensor_copy(kstk[64:128, d0 : d0 + 256], kp[64:128, 0:256])
            nc.vector.tensor_copy(kstk[64:128, d0 + 256 : d0 + 512], kp[64:128, 256:512])

        s_tiles = {}
        pt_tiles = {}

        def st_mms(uname, tiers):
            """emit this unit's S^T matmuls whose tier is in `tiers`."""
            name_u = uname
            if name_u not in s_tiles:
                s_tiles[name_u] = psS.tile(
                    [128, 4, 256], fp32, tag="s", name=f"sps_{name_u}"
                )
            sp = s_tiles[name_u]
            udef = next(u for u in units if u[0] == name_u)
            slices = udef[1]
            halves = udef[2]

            def slice_tier(sl):
                for lo, hi, t in halves:
                    if lo <= sl < hi:
                        return t
                raise AssertionError

            for (slo, nsl, side, ch, slot_lo) in _ST_MMS[name_u]:
                if slice_tier(slo) not in tiers:
                    continue
                rl = slice(64 * side, 64 * side + 64)
                nc.tensor.matmul(
                    sp[:, slo : slo + nsl, :].rearrange("p a q -> p (a q)"),
                    lhsT=kstk[rl, ch * CHUNK : (ch + 1) * CHUNK],
                    rhs=qT[rl, slot_lo * QTILE : (slot_lo + nsl) * QTILE],
                    start=True, stop=True,
                )

        def exp_half(uname, lo, hi):
            sp = s_tiles[uname]
            if uname not in pt_tiles:
                pt_tiles[uname] = ptpool.tile(
                    [128, 4, 256], bf16, tag="pt", name=f"pt_{uname}"
                )
            pt = pt_tiles[uname]
            nc.scalar.activation(
                pt[:, lo:hi, :].rearrange("p a q -> p (a q)"),
                sp[:, lo:hi, :].rearrange("p a q -> p (a q)"),
                EXP, scale=SCALE,
            )

        def diag_mask(uname, slo):
            pt = pt_tiles[uname]
            nc.gpsimd.affine_select(
                out=pt[:, slo : slo + 2, :],
                in_=pt[:, slo : slo + 2, :],
                compare_op=mybir.AluOpType.is_ge,
                fill=0.0,
                base=0,
                # keep where f - p - 128a >= 0  <=>  key p+128a <= query f
                pattern=[[-128, 2], [1, 256]],
                channel_multiplier=-1,
            )

        def rmask(uname, slo, nsl):
            pt = pt_tiles[uname]
            nc.vector.tensor_scalar_mul(
                pt[:, slo : slo + nsl, :].rearrange("p a q -> p (a q)"),
                pt[:, slo : slo + nsl, :].rearrange("p a q -> p (a q)"),
                scal_sb[:, 0:1],
            )

        def transp(c):
            """V (natural) for own chunk c and oth chunk c via one 128x128
            transpose of kstk column range c."""
            tp = psP.tile([128, 1024], bf16, tag="proj", name=f"tp{c}")
            nc.tensor.transpose(
                tp[:, 0:128],
                in_=kstk[:, c * CHUNK : (c + 1) * CHUNK],
                identity=ident,
            )
            nc.vector.tensor_copy(
                vones[:, c, :, 0:D],
                tp[:, 0:128].rearrange("p (a d) -> p a d", a=2),
            )

        o_ps_holder = {}
        pv_seen = {}
        o_init_done = {}
        pv_total = {}

        def pv_count():
            """precompute per-region PV totals from unit defs."""
            for u in units:
                for (side, ch, slot) in u[1]:
                    pv_total[slot] = pv_total.get(slot, 0) + 1

        pv_count()

        def pv(uname, which=None):
            """PV matmuls for unit's cells (which: filter by slice index)."""
            if "o" not in o_ps_holder:
                o_ps_holder["o"] = psO.tile(
                    [D + 1, N_SLOTS * QTILE], fp32, name="o_ps"
                )
                # one start=True zero-matmul per 512-col PSUM bank: the ONLY
                # start in each bank (start clears has_written bank-wide, so
                # interleaved per-region starts would drop accumulation).
                for bank in range(2):
                    nc.tensor.matmul(
                        o_ps_holder["o"][:, bank * 512 : (bank + 1) * 512],
                        lhsT=warm_src[:, 0 : D + 1],
                        rhs=warm_src,
                        start=True, stop=False, skip_group_check=True,
                    )
            o_ps = o_ps_holder["o"]
            udef = next(u for u in units if u[0] == uname)
            pt = pt_tiles[uname]
            for sl, (side, ch, slot) in enumerate(udef[1]):
                if which is not None and sl not in which:
                    continue
                seen = pv_seen.get(slot, 0)
                pv_seen[slot] = seen + 1
                nc.tensor.matmul(
                    o_ps[:, slot * QTILE : (slot + 1) * QTILE],
                    lhsT=vones[:, ch, side, 0 : D + 1],
                    rhs=pt[:, sl, :],
                    start=False,
                    stop=(seen + 1 == pv_total[slot]),
                    skip_group_check=True,
                )

        def close_region(j):
            o_ps = o_ps_holder["o"]
            o_sb = opool.tile([D + 1, QTILE], fp32, name=f"osb{j}")
            nc.vector.tensor_copy(
                o_sb, o_ps[:, j * QTILE : (j + 1) * QTILE]
            )
            nc.sync.dma_start(out=out[:, j * QTILE : (j + 1) * QTILE], in_=o_sb)

        # ---- emission schedule (queue order == dependency order) ----
        # tier 0: s0a
        kq0a = joint_proj(0, 0, 256)
        qlow_proj(0, 0, 256)
        st_mms("W1", {0})
        exp_half("W1", 0, 2)
        diag_mask("W1", 0)
        qthi(kq0a, 0, 0, 256)
        # tier 1: s0b -- W1h2 needs only qlow0b; W2h1 needs joint0b kstk
        qlow_proj(0, 256, 512)
        st_mms("W1", {1})
        exp_half("W1", 2, 4)
        kq0b = joint_proj(0, 256, 512)
        st_mms("W2", {1})
        exp_half("W2", 0, 2)
        diag_mask("W2", 0)
        qthi(kq0b, 0, 256, 512)
        # tier 2: s1 -- W2h2/W4 need only qlow1; W3/W5 need joint1 kstk
        qlow_proj(1, 0, 512)
        st_mms("W2", {2})
        st_mms("W4", {2})
        exp_half("W2", 2, 4)
        exp_half("W4", 0, 4)
        kq1 = joint_proj(1, 0, 512)
        st_mms("W3", {2})
        st_mms("W5", {2})
        exp_half("W3", 0, 4)
        diag_mask("W3", 2)
        exp_half("W5", 0, 4)
        diag_mask("W5", 2)
        qthi(kq1, 1, 0, 512)
        # tier 3: s2
        kproj_oth(2)
        st_mms("W6", {3})
        st_mms("W7", {3})
        st_mms("W8", {3})
        st_mms("W10", {3})
        exp_half("W6", 0, 4)
        exp_half("W7", 0, 4)
        exp_half("W8", 0, 4)
        exp_half("W10", 0, 2)
        transp(0)
        transp(1)
        transp(2)
        transp(3)
        rmask("W6", 0, 1)
        rmask("W6", 2, 1)
        pv("W1")
        pv("W2")
        pv("W6")
        close_region(0)
        # tier 4: s3 -- exps first, minimal tail after the last exp
        kproj_oth(3)
        st_mms("W9", {4})
        exp_half("W9", 0, 4)
        st_mms("W10", {4})
        exp_half("W10", 2, 4)
        transp(4)
        transp(5)
        rmask("W8", 0, 1)
        rmask("W8", 2, 1)
        pv("W3")
        pv("W7")
        pv("W8")
        close_region(1)
        rmask("W9", 0, 1)
        rmask("W9", 2, 1)
        pv("W9")
        close_region(2)
        transp(6)
        transp(7)
        pv("W4")
        pv("W10", which={0, 1})
        rmask("W10", 2, 2)
        pv("W5")
        pv("W10", which={2, 3})
        close_region(3)

    nc.compile()
    return nc


_NC_CACHE = None


def _get_nc():
    global _NC_CACHE
    if _NC_CACHE is None:
        _NC_CACHE = _build_graph()
    return _NC_CACHE


def _perm_tiles(r):
    own = [2 * j + r for j in range(N_SLOTS)]
    oth = [2 * j + (1 - r) for j in range(N_SLOTS)]
    return own + oth


def _host_prep(x, W_Q, W_K):
    in_maps = []
    wkq = np.concatenate([W_K.T, W_Q.T], axis=1).astype(BF16)  # [1024, 128]
    for i in range(N_CORES):
        b, r = i % B, i // B
        perm = _perm_tiles(r)
        xt = x[b].T.astype(BF16)  # [1024, 2048]
        cols = np.concatenate(
            [np.arange(QTILE * p, QTILE * p + QTILE) for p in perm]
        )
        xkt = np.ascontiguousarray(xt[:, cols])
        sc = np.full((CHUNK, 1), float(r), dtype=np.float32)
        in_maps.append({"xkt": xkt, "wkq": wkq, "scal": sc})
    return in_maps


def _ensure_ntff_hook():
    import types

    try:
        from antenv.axon_hooks import get_axon_ntff_profile_hook  # noqa: F401

        return
    except ImportError:
        pass
    import antenv

    mod = types.ModuleType("antenv.axon_hooks")
    mod._hook = None

    def set_axon_ntff_profile_hook(h):
        mod._hook = h

    def get_axon_ntff_profile_hook():
        return mod._hook

    mod.set_axon_ntff_profile_hook = set_axon_ntff_profile_hook
    mod.get_axon_ntff_profile_hook = get_axon_ntff_profile_hook
    sys.modules["antenv.axon_hooks"] = mod
    antenv.axon_hooks = mod
    try:
        from trn_agent_boot.trn_boot import _ntff_profile_via_ctypes

        hook = _ntff_profile_via_ctypes("/opt/axon/libaxon_pjrt.so")
        if hook is not None:
            set_axon_ntff_profile_hook(hook)
    except Exception as e:
        print(f"ntff hook install failed: {e}")


def kernel(x, W_Q, W_K, W_V=None, **_unused):
    global LAST_RESULTS
    if TRACE:
        _ensure_ntff_hook()
    x = np.asarray(x, dtype=np.float32)
    W_Q = np.asarray(W_Q, dtype=np.float32)
    W_K = np.asarray(W_K, dtype=np.float32)

    from concourse.bass_utils import run_bass_kernel_spmd

    nc = _get_nc()
    in_maps = _host_prep(x, W_Q, W_K)
    res = run_bass_kernel_spmd(
        nc,
        in_maps,
        core_ids=list(range(N_CORES)),
        trace=TRACE,
        trace_cores=TRACE_CORES,
    )
    LAST_RESULTS = res

    y = np.empty((B, T, D), dtype=np.float32)
    for i in range(N_CORES):
        b, r = i % B, i // B
        ot = res.results[i]["out"]  # [65, 1024]
        o = ot[0:D, :] / ot[D : D + 1, :]
        for j in range(N_SLOTS):
            t0 = QTILE * (2 * j + r)
            y[b, t0 : t0 + QTILE, :] = o[:, j * QTILE : (j + 1) * QTILE].T
    return y


# revision 13
# speedup vs baseline: 1.0462x; 1.0462x over previous
"""Single-head causal attention (V=K source bug) on 8 trn2 NeuronCores. v2.

Problem: x[4,2048,1024], W_Q/W_K/W_V[64,1024] (W_V unused by reference).
  Q = x @ W_Q.T ; K = x @ W_K.T ; V = K (reference bug)
  out = softmax(mask(Q K^T / sqrt(1024))) @ V      -> [4,2048,64]

Sharding: 2 cores per batch (core i: batch = i % 4, role r = i // 4).
Each batch's 8 query tiles of 256 rows split by parity (r=0 even, r=1 odd).
ONE SPMD graph; per-core differences folded into data:
 * x^T sent column-PERMUTED (own tiles first); causality over the permuted
   key order is uniform: own chunk 2s+h at slot s masks p+128h<=f
   (device affine_select, no mask DMA); oth chunks (2s,2s+1) at slot s
   are all-valid (r=1) or all-masked (r=0) -> scal 0/1 multiply.

v2 layout (vs v1): single sync HWDGE queue, dep-ordered col-half DMAs;
qT duplicated into rows 0-63 (qlow pass) and 64-127 (joint K|Q pass,
col-tiled) with no SBUF-SBUF dup DMA; kstk rows 0-63 = own K^T chunks,
rows 64-127 = oth (tile_position col packing); one [128,128] PE transpose
per chunk column yields V for own+oth together; outputs on the sync HW
queue. ScalarE runs only the 13-ACT exp chain.
"""

import sys

sys.path.insert(0, "/opt/trn_rl_repo")

import numpy as np
import ml_dtypes

BF16 = ml_dtypes.bfloat16

B, T, C, D = 4, 2048, 1024, 64
N_CORES = 8
QTILE = 256
N_SLOTS = 4
CHUNK = 128
SCALE = C ** -0.5
N_WARMUP = 32

TRACE = False
TRACE_CORES = None
LAST_RESULTS = None

# ---- unit table ----------------------------------------------------------
# cell = (side, chunk, slot): side 0 = own (kstk rows 0:64, rhs qT[0:64]),
# side 1 = oth (rows 64:128, rhs qT[64:128]).
# Units W1..W10: 4 slices of [128,256] in one PSUM tile [128, 4, 256].
# Each entry: (name, slices[4] of cell, exp_halves, diag, rmask)
#   exp_halves: list of (slice_lo, slice_hi, tier) - ACTIVATE granularity
#   diag: (slice_lo,) one affine_select over slices [lo, lo+2) or None
#   rmask: list of (slice_lo, n_slices) tensor_scalar muls
# tiers (DMA arrival): 0=s0a 1=s0b 2=s1 3=s2 4=s3


def _unit_defs():
    U = []
    U.append(("W1", [(0, 0, 0), (0, 1, 0), (0, 0, 1), (0, 1, 1)],
              [(0, 2, 0), (2, 4, 1)], 0, []))
    U.append(("W2", [(0, 2, 1), (0, 3, 1), (0, 2, 2), (0, 3, 2)],
              [(0, 2, 1), (2, 4, 2)], 0, []))
    U.append(("W3", [(0, 0, 2), (0, 1, 2), (0, 4, 2), (0, 5, 2)],
              [(0, 4, 2)], 2, []))
    U.append(("W4", [(0, 0, 3), (0, 1, 3), (0, 2, 3), (0, 3, 3)],
              [(0, 4, 2)], None, []))
    U.append(("W5", [(0, 4, 3), (0, 5, 3), (0, 6, 3), (0, 7, 3)],
              [(0, 4, 2)], 2, []))
    U.append(("W6", [(1, 0, 0), (1, 0, 1), (1, 1, 0), (1, 1, 1)],
              [(0, 4, 3)], None, [(0, 1), (2, 1)]))
    U.append(("W7", [(1, 0, 2), (1, 0, 3), (1, 1, 2), (1, 1, 3)],
              [(0, 4, 3)], None, []))
    U.append(("W8", [(1, 2, 1), (1, 2, 2), (1, 3, 1), (1, 3, 2)],
              [(0, 4, 3)], None, [(0, 1), (2, 1)]))
    U.append(("W9", [(1, 4, 2), (1, 4, 3), (1, 5, 2), (1, 5, 3)],
              [(0, 4, 4)], None, [(0, 1), (2, 1)]))
    U.append(("W10", [(1, 2, 3), (1, 3, 3), (1, 6, 3), (1, 7, 3)],
              [(0, 2, 3), (2, 4, 4)], None, [(2, 2)]))
    return U


# S^T matmuls per unit: list of (slice_lo, n_slices, side, chunk, slot_lo)
# wide (n_slices=2) only when the slices are the same chunk at adjacent
# slots AND arrive in one tier.
_ST_MMS = {
    "W1": [(0, 1, 0, 0, 0), (1, 1, 0, 1, 0), (2, 1, 0, 0, 1), (3, 1, 0, 1, 1)],
    "W2": [(0, 1, 0, 2, 1), (1, 1, 0, 3, 1), (2, 1, 0, 2, 2), (3, 1, 0, 3, 2)],
    "W3": [(0, 1, 0, 0, 2), (1, 1, 0, 1, 2), (2, 1, 0, 4, 2), (3, 1, 0, 5, 2)],
    "W4": [(0, 1, 0, 0, 3), (1, 1, 0, 1, 3), (2, 1, 0, 2, 3), (3, 1, 0, 3, 3)],
    "W5": [(0, 1, 0, 4, 3), (1, 1, 0, 5, 3), (2, 1, 0, 6, 3), (3, 1, 0, 7, 3)],
    "W6": [(0, 2, 1, 0, 0), (2, 2, 1, 1, 0)],
    "W7": [(0, 2, 1, 0, 2), (2, 2, 1, 1, 2)],
    "W8": [(0, 2, 1, 2, 1), (2, 2, 1, 3, 1)],
    "W9": [(0, 2, 1, 4, 2), (2, 2, 1, 5, 2)],
    "W10": [(0, 1, 1, 2, 3), (1, 1, 1, 3, 3), (2, 1, 1, 6, 3), (3, 1, 1, 7, 3)],
}


def _build_graph():
    import concourse.bass as bass
    import concourse.mybir as mybir
    import concourse.tile as tile
    from concourse import bacc
    from concourse.masks import make_identity
    from contextlib import ExitStack

    fp32 = mybir.dt.float32
    bf16 = mybir.dt.bfloat16
    EXP = mybir.ActivationFunctionType.Exp

    nc = bacc.Bacc(
        "TRN2",
        target_bir_lowering=False,
        debug=False,
        num_devices=N_CORES,
    )

    # host pre-arranges inputs partition-major so every DMA line is
    # 2-4KB contiguous per partition (256B/1KB lines cost ~25% DMA rate).
    xk = nc.dram_tensor(
        "xk", [8, CHUNK, C // CHUNK, 256], bf16, kind="ExternalInput"
    ).ap()
    wkq = nc.dram_tensor(
        "wkq", [CHUNK, C // CHUNK, 2 * D], bf16, kind="ExternalInput"
    ).ap()
    scald = nc.dram_tensor("scal", [CHUNK, 1], fp32, kind="ExternalInput").ap()
    out = nc.dram_tensor(
        "out", [D + 1, N_SLOTS * QTILE], fp32, kind="ExternalOutput"
    ).ap()

    CCH = C // CHUNK  # 8 contraction chunks
    assert xk.shape[2] == CCH
    units = _unit_defs()

    with tile.TileContext(nc) as tc, ExitStack() as ctx:
        consts = ctx.enter_context(tc.tile_pool(name="consts", bufs=1))
        xpool = ctx.enter_context(tc.tile_pool(name="xpool", bufs=1))
        kqpool = ctx.enter_context(tc.tile_pool(name="kqpool", bufs=1))
        ptpool = ctx.enter_context(tc.tile_pool(name="ptpool", bufs=10))
        opool = ctx.enter_context(tc.tile_pool(name="opool", bufs=4))
        psS = ctx.enter_context(tc.tile_pool(name="psS", bufs=2, space="PSUM"))
        psP = ctx.enter_context(tc.tile_pool(name="psP", bufs=2, space="PSUM"))
        psO = ctx.enter_context(tc.tile_pool(name="psO", bufs=1, space="PSUM"))

        # ---- constants / statics ----
        warm_src = consts.tile([128, 512], bf16)
        nc.vector.memset(warm_src, 0.0)
        warm_ps = psP.tile([128, 512], fp32, tag="proj", name="warm_ps")
        for w in range(N_WARMUP):
            nc.tensor.matmul(
                warm_ps[:, 0:128], lhsT=warm_src[:, 0:128],
                rhs=warm_src[:, 0:128],
                start=(w == 0), stop=(w == N_WARMUP - 1),
            )
        ident = consts.tile([128, 128], bf16)
        make_identity(nc, ident)
        warm = consts.tile([1, 1], fp32)
        nc.vector.memset(warm, 0.0)
        nc.scalar.activation(warm, warm, EXP)

        w_sb = consts.tile([128, CCH, 2 * D], bf16)
        scal_sb = consts.tile([128, 1], fp32)
        xs = [xpool.tile([128, CCH, 512], bf16, name=f"xslab{s}")
              for s in range(4)]

        # ---- input DMAs: one sync HWDGE queue, dependency order ----
        nc.sync.dma_start(out=scal_sb, in_=scald)
        nc.sync.dma_start(out=w_sb, in_=wkq)
        for b in range(8):
            s, h = b // 2, b % 2
            nc.sync.dma_start(
                out=xs[s][:, :, h * 256 : h * 256 + 256], in_=xk[b]
            )

        # ---- persistent SBUF tensors ----
        kstk = kqpool.tile([128, 1024], bf16)   # r0-63 own K^T, r64-127 oth
        qT = kqpool.tile([128, 1024], bf16)     # Q^T in both halves
        vones = kqpool.tile([128, CCH, 2, D + 2], bf16)
        nc.vector.memset(vones[:, :, :, D : D + 1], 1.0)

        # ---- engine-op emitters ----
        def joint_proj(s, c0, c1):
            """K|Q joint projection of cols [c0,c1) of slab s (slabs 0,1).
            rows 0-63 K^T -> kstk, rows 64-127 Q^T -> qT hi (both DVE)."""
            w = c1 - c0
            kq = psP.tile([128, 512], fp32, tag="proj", name=f"kq{s}{c0}")
            for c in range(CCH):
                nc.tensor.matmul(
                    kq[:, 0:w], lhsT=w_sb[:, c, :], rhs=xs[s][:, c, c0:c1],
                    start=(c == 0), stop=(c == CCH - 1),
                )
            dst = slice(s * 512 + c0, s * 512 + c1)
            nc.vector.tensor_copy(kstk[0:64, dst], kq[0:64, 0:w])
            return kq

        def qthi(kq, s, c0, c1):
            """deferred Q^T hi copy (only oth-side S^T needs it)."""
            w = c1 - c0
            nc.vector.tensor_copy(
                qT[64:128, s * 512 + c0 : s * 512 + c1], kq[64:128, 0:w]
            )

        def qlow_proj(s, c0, c1):
            w = c1 - c0
            qp = psP.tile([128, 512], fp32, tag="proj", name=f"ql{s}{c0}")
            for c in range(CCH):
                nc.tensor.matmul(
                    qp[0:64, 0:w], lhsT=w_sb[:, c, D : 2 * D],
                    rhs=xs[s][:, c, c0:c1],
                    start=(c == 0), stop=(c == CCH - 1),
                )
            dst = slice(s * 512 + c0, s * 512 + c1)
            nc.vector.tensor_copy(qT[0:64, dst], qp[0:64, 0:w])

        def kproj_oth(s):
            """K^T of full slab s (2,3) -> kstk rows 64-127 (col-tiled via
            out base_partition 64)."""
            kp = psP.tile([128, 512], fp32, tag="proj", name=f"ko{s}")
            for c in range(CCH):
                nc.tensor.matmul(
                    kp[64:128, :], lhsT=w_sb[:, c, 0:D], rhs=xs[s][:, c, :],
                    start=(c == 0), stop=(c == CCH - 1),
                )
            d0 = (s - 2) * 512
            nc.vector.tensor_copy(kstk[64:128, d0 : d0 + 256], kp[64:128, 0:256])
            nc.vector.tensor_copy(kstk[64:128, d0 + 256 : d0 + 512], kp[64:128, 256:512])

        s_tiles = {}
        pt_tiles = {}

        def st_mms(uname, tiers):
            """emit this unit's S^T matmuls whose tier is in `tiers`."""
            name_u = uname
            if name_u not in s_tiles:
                s_tiles[name_u] = psS.tile(
                    [128, 4, 256], fp32, tag="s", name=f"sps_{name_u}"
                )
            sp = s_tiles[name_u]
            udef = next(u for u in units if u[0] == name_u)
            slices = udef[1]
            halves = udef[2]

            def slice_tier(sl):
                for lo, hi, t in halves:
                    if lo <= sl < hi:
                        return t
                raise AssertionError

            for (slo, nsl, side, ch, slot_lo) in _ST_MMS[name_u]:
                if slice_tier(slo) not in tiers:
                    continue
                rl = slice(64 * side, 64 * side + 64)
                nc.tensor.matmul(
                    sp[:, slo : slo + nsl, :].rearrange("p a q -> p (a q)"),
                    lhsT=kstk[rl, ch * CHUNK : (ch + 1) * CHUNK],
                    rhs=qT[rl, slot_lo * QTILE : (slot_lo + nsl) * QTILE],
                    start=True, stop=True,
                )

        def exp_half(uname, lo, hi):
            sp = s_tiles[uname]
            if uname not in pt_tiles:
                pt_tiles[uname] = ptpool.tile(
                    [128, 4, 256], bf16, tag="pt", name=f"pt_{uname}"
                )
            pt = pt_tiles[uname]
            nc.scalar.activation(
                pt[:, lo:hi, :].rearrange("p a q -> p (a q)"),
                sp[:, lo:hi, :].rearrange("p a q -> p (a q)"),
                EXP, scale=SCALE,
            )

        def diag_mask(uname, slo):
            pt = pt_tiles[uname]
            nc.gpsimd.affine_select(
                out=pt[:, slo : slo + 2, :],
                in_=pt[:, slo : slo + 2, :],
                compare_op=mybir.AluOpType.is_ge,
                fill=0.0,
                base=0,
                # keep where f - p - 128a >= 0  <=>  key p+128a <= query f
                pattern=[[-128, 2], [1, 256]],
                channel_multiplier=-1,
            )

        def rmask(uname, slo, nsl):
            pt = pt_tiles[uname]
            nc.vector.tensor_scalar_mul(
                pt[:, slo : slo + nsl, :].rearrange("p a q -> p (a q)"),
                pt[:, slo : slo + nsl, :].rearrange("p a q -> p (a q)"),
                scal_sb[:, 0:1],
            )

        def transp(c):
            """V (natural) for own chunk c and oth chunk c via one 128x128
            transpose of kstk column range c."""
            tp = psP.tile([128, 1024], bf16, tag="proj", name=f"tp{c}")
            nc.tensor.transpose(
                tp[:, 0:128],
                in_=kstk[:, c * CHUNK : (c + 1) * CHUNK],
                identity=ident,
            )
            nc.vector.tensor_copy(
                vones[:, c, :, 0:D],
                tp[:, 0:128].rearrange("p (a d) -> p a d", a=2),
            )

        o_ps_holder = {}
        pv_seen = {}
        o_init_done = {}
        pv_total = {}

        def pv_count():
            """precompute per-region PV totals from unit defs."""
            for u in units:
                for (side, ch, slot) in u[1]:
                    pv_total[slot] = pv_total.get(slot, 0) + 1

        pv_count()

        def pv(uname, which=None):
            """PV matmuls for unit's cells (which: filter by slice index)."""
            if "o" not in o_ps_holder:
                o_ps_holder["o"] = psO.tile(
                    [D + 1, N_SLOTS * QTILE], fp32, name="o_ps"
                )
                # one start=True zero-matmul per 512-col PSUM bank: the ONLY
                # start in each bank (start clears has_written bank-wide, so
                # interleaved per-region starts would drop accumulation).
                for bank in range(2):
                    nc.tensor.matmul(
                        o_ps_holder["o"][:, bank * 512 : (bank + 1) * 512],
                        lhsT=warm_src[:, 0 : D + 1],
                        rhs=warm_src,
                        start=True, stop=False, skip_group_check=True,
                    )
            o_ps = o_ps_holder["o"]
            udef = next(u for u in units if u[0] == uname)
            pt = pt_tiles[uname]
            for sl, (side, ch, slot) in enumerate(udef[1]):
                if which is not None and sl not in which:
                    continue
                seen = pv_seen.get(slot, 0)
                pv_seen[slot] = seen + 1
                nc.tensor.matmul(
                    o_ps[:, slot * QTILE : (slot + 1) * QTILE],
                    lhsT=vones[:, ch, side, 0 : D + 1],
                    rhs=pt[:, sl, :],
                    start=False,
                    stop=(seen + 1 == pv_total[slot]),
                    skip_group_check=True,
                )

        def close_region(j):
            o_ps = o_ps_holder["o"]
            o_sb = opool.tile([D + 1, QTILE], fp32, name=f"osb{j}")
            nc.vector.tensor_copy(
                o_sb, o_ps[:, j * QTILE : (j + 1) * QTILE]
            )
            nc.sync.dma_start(out=out[:, j * QTILE : (j + 1) * QTILE], in_=o_sb)

        # ---- emission schedule (queue order == dependency order) ----
        # tier 0: s0a
        kq0a = joint_proj(0, 0, 256)
        qlow_proj(0, 0, 256)
        st_mms("W1", {0})
        exp_half("W1", 0, 2)
        diag_mask("W1", 0)
        qthi(kq0a, 0, 0, 256)
        # tier 1: s0b -- W1h2 needs only qlow0b; W2h1 needs joint0b kstk
        qlow_proj(0, 256, 512)
        st_mms("W1", {1})
        exp_half("W1", 2, 4)
        kq0b = joint_proj(0, 256, 512)
        st_mms("W2", {1})
        exp_half("W2", 0, 2)
        diag_mask("W2", 0)
        qthi(kq0b, 0, 256, 512)
        # tier 2: s1 -- W2h2/W4 need only qlow1; W3/W5 need joint1 kstk
        qlow_proj(1, 0, 512)
        st_mms("W2", {2})
        st_mms("W4", {2})
        exp_half("W2", 2, 4)
        exp_half("W4", 0, 4)
        kq1 = joint_proj(1, 0, 512)
        st_mms("W3", {2})
        st_mms("W5", {2})
        exp_half("W3", 0, 4)
        diag_mask("W3", 2)
        exp_half("W5", 0, 4)
        diag_mask("W5", 2)
        qthi(kq1, 1, 0, 512)
        # tier 3: s2
        kproj_oth(2)
        st_mms("W6", {3})
        st_mms("W7", {3})
        st_mms("W8", {3})
        st_mms("W10", {3})
        exp_half("W6", 0, 4)
        exp_half("W7", 0, 4)
        exp_half("W8", 0, 4)
        exp_half("W10", 0, 2)
        transp(0)
        transp(1)
        transp(2)
        transp(3)
        rmask("W6", 0, 1)
        rmask("W6", 2, 1)
        pv("W1")
        pv("W2")
        pv("W6")
        close_region(0)
        # tier 4: s3 -- exps first, minimal tail after the last exp
        kproj_oth(3)
        st_mms("W9", {4})
        exp_half("W9", 0, 4)
        st_mms("W10", {4})
        exp_half("W10", 2, 4)
        transp(4)
        transp(5)
        rmask("W8", 0, 1)
        rmask("W8", 2, 1)
        pv("W3")
        pv("W7")
        pv("W8")
        close_region(1)
        rmask("W9", 0, 1)
        rmask("W9", 2, 1)
        pv("W9")
        close_region(2)
        transp(6)
        transp(7)
        pv("W4")
        pv("W10", which={0, 1})
        rmask("W10", 2, 2)
        pv("W5")
        pv("W10", which={2, 3})
        close_region(3)

    nc.compile()
    return nc


_NC_CACHE = None


def _get_nc():
    global _NC_CACHE
    if _NC_CACHE is None:
        _NC_CACHE = _build_graph()
    return _NC_CACHE


def _perm_tiles(r):
    own = [2 * j + r for j in range(N_SLOTS)]
    oth = [2 * j + (1 - r) for j in range(N_SLOTS)]
    return own + oth


def _host_prep(x, W_Q, W_K):
    in_maps = []
    CCH = C // CHUNK
    wkq2 = np.concatenate([W_K.T, W_Q.T], axis=1).astype(BF16)  # [1024, 128]
    # [128, 8, 128]: wkq_pm[p, c, d] = wkq2[c*128 + p, d]
    wkq_pm = np.ascontiguousarray(wkq2.reshape(CCH, CHUNK, 2 * D).transpose(1, 0, 2))
    for i in range(N_CORES):
        b, r = i % B, i // B
        perm = _perm_tiles(r)
        xt = x[b].T.astype(BF16)  # [1024, 2048]
        cols = np.concatenate(
            [np.arange(QTILE * p, QTILE * p + QTILE) for p in perm]
        )
        xkt = xt[:, cols].reshape(CCH, CHUNK, T)  # [c, p, t]
        # 8 blocks [128, 8, 256]: block b2 = cols [b2*256, +256), partition-major
        xkb = np.ascontiguousarray(
            xkt.transpose(1, 0, 2)
            .reshape(CHUNK, CCH, 8, 256)
            .transpose(2, 0, 1, 3)
        )
        sc = np.full((CHUNK, 1), float(r), dtype=np.float32)
        in_maps.append({"xk": xkb, "wkq": wkq_pm, "scal": sc})
    return in_maps


def _ensure_ntff_hook():
    import types

    try:
        from antenv.axon_hooks import get_axon_ntff_profile_hook  # noqa: F401

        return
    except ImportError:
        pass
    import antenv

    mod = types.ModuleType("antenv.axon_hooks")
    mod._hook = None

    def set_axon_ntff_profile_hook(h):
        mod._hook = h

    def get_axon_ntff_profile_hook():
        return mod._hook

    mod.set_axon_ntff_profile_hook = set_axon_ntff_profile_hook
    mod.get_axon_ntff_profile_hook = get_axon_ntff_profile_hook
    sys.modules["antenv.axon_hooks"] = mod
    antenv.axon_hooks = mod
    try:
        from trn_agent_boot.trn_boot import _ntff_profile_via_ctypes

        hook = _ntff_profile_via_ctypes("/opt/axon/libaxon_pjrt.so")
        if hook is not None:
            set_axon_ntff_profile_hook(hook)
    except Exception as e:
        print(f"ntff hook install failed: {e}")


def kernel(x, W_Q, W_K, W_V=None, **_unused):
    global LAST_RESULTS
    if TRACE:
        _ensure_ntff_hook()
    x = np.asarray(x, dtype=np.float32)
    W_Q = np.asarray(W_Q, dtype=np.float32)
    W_K = np.asarray(W_K, dtype=np.float32)

    from concourse.bass_utils import run_bass_kernel_spmd

    nc = _get_nc()
    in_maps = _host_prep(x, W_Q, W_K)
    res = run_bass_kernel_spmd(
        nc,
        in_maps,
        core_ids=list(range(N_CORES)),
        trace=TRACE,
        trace_cores=TRACE_CORES,
    )
    LAST_RESULTS = res

    y = np.empty((B, T, D), dtype=np.float32)
    for i in range(N_CORES):
        b, r = i % B, i // B
        ot = res.results[i]["out"]  # [65, 1024]
        o = ot[0:D, :] / ot[D : D + 1, :]
        for j in range(N_SLOTS):
            t0 = QTILE * (2 * j + r)
            y[b, t0 : t0 + QTILE, :] = o[:, j * QTILE : (j + 1) * QTILE].T
    return y
